# revision 4
# baseline (speedup 1.0000x reference)
"""nn_ConvP4 TRN2 Bass kernel: P4 group-equivariant convolution on 8 NeuronCores.

out[b,j,h,w,f] = sum_{k,a,v,c} x[b,(j+k-1)%4, h+a-1, w+v-1, c] * rot90(kernel,j)[a,v,k,c,f]
  x: [8,4,128,128,64] f32, kernel: [3,3,3,64,128] f32 -> out: [8,4,128,128,128] f32

Sharding: data-parallel over batch, one batch item per core (8 cores), kernel
weights replicated. No cross-device communication.

Device algorithm (per core / batch item): for each output group j and each
4-row output quad, accumulate 14 matmuls [K=128, M=128(F)] x [K=128, N=512]
into one PSUM bank. The 27 taps (3 kh x 3 kw x 3 group-depth) are packed two
per matmul by stacking two tap-shifted copies of x in the 128 SBUF partitions
(64 channels each):
  - 9 slots: taps (a,v,k=0) + (a,v,k=1) stacked channel-wise (tensor P0).
  - 3 slots: taps (0,v,2) + (1,v,2) via a row-shifted staging tensor P1.
  - 1 slot:  taps (2,0,2) + (2,1,2) via a column-shifted staging tensor P2.
  - 1 slot:  tap (2,2,2) alone (27 is odd), zero weights in the high half.
Inputs and weights are staged in bf16 (halves DMA traffic, full-rate PE);
accumulation is fp32 in PSUM. Measured rel error ~2.3e-3 for this 1728-term
contraction, well under the 2e-2 gate.

Host side stages channel-major, spatially zero-padded tensors so the device
only ever issues dense, large, contiguous-per-partition DMAs, and the output
is produced in [j, f, h, w] layout which the host transposes during unshard.
"""

from contextlib import ExitStack

import ml_dtypes
import numpy as np

import concourse.bacc as bacc
import concourse.tile as tile
from concourse import mybir
from concourse.bass_utils import run_bass_kernel_spmd

N_CORES = 8
B = 8
H = W = 128
CIN = 64
F = 128
HP = 132  # padded rows in staged tensors
WP = 130  # padded cols
NQ = 4  # quads per PSUM group
GROUPS = H // (4 * NQ)
NS = 14  # matmul slots per quad

F32 = mybir.dt.float32
BF16 = mybir.dt.bfloat16
NP_BF16 = ml_dtypes.bfloat16


def _stage_inputs(x: np.ndarray, kern: np.ndarray):
    """Per-core input maps from full inputs."""
    xt = np.ascontiguousarray(x.transpose(0, 1, 4, 2, 3))  # [b,g,c,h,w]
    xt = xt.astype(NP_BF16)
    P0 = np.zeros((B, 4, 128, HP, WP), NP_BF16)
    P1 = np.zeros((B, 4, 128, HP, WP), NP_BF16)
    P2 = np.zeros((B, 4, 128, HP, WP), NP_BF16)
    for t in range(4):
        P0[:, t, 0:64, 1 : H + 1, 1 : W + 1] = xt[:, t]
        P0[:, t, 64:128, 1 : H + 1, 1 : W + 1] = xt[:, (t + 1) % 4]
        P1[:, t, 0:64, 2 : H + 2, 1 : W + 1] = xt[:, t]
        P1[:, t, 64:128, 1 : H + 1, 1 : W + 1] = xt[:, t]
        P2[:, t, 0:64, 2 : H + 2, 1 : W + 1] = xt[:, t]
        P2[:, t, 64:128, 2 : H + 2, 0:W] = xt[:, t]

    Wpk = np.zeros((4, NS, 128, F), np.float32)
    for j in range(4):
        Kj = np.rot90(kern, k=j, axes=(0, 1))
        for a in range(3):
            for v in range(3):
                Wpk[j, 3 * a + v, 0:64] = Kj[a, v, 0]
                Wpk[j, 3 * a + v, 64:128] = Kj[a, v, 1]
        for v in range(3):
            Wpk[j, 9 + v, 0:64] = Kj[0, v, 2]
            Wpk[j, 9 + v, 64:128] = Kj[1, v, 2]
        Wpk[j, 12, 0:64] = Kj[2, 0, 2]
        Wpk[j, 12, 64:128] = Kj[2, 1, 2]
        Wpk[j, 13, 0:64] = Kj[2, 2, 2]
        Wpk[j, 13, 64:128] = Kj[2, 2, 2]  # duplicated for the high row-tile
    Wpk = Wpk.astype(NP_BF16)
    return [{"p0": P0[b], "p1": P1[b], "p2": P2[b], "wt": Wpk} for b in range(B)]


def build_program(loop_iters: int = 1, out_bufs: int = 6, win_bufs: int = 3):
    """Build + compile the per-core Bass program (identical on all cores)."""
    nc = bacc.Bacc("TRN2", target_bir_lowering=False, debug=False, num_devices=N_CORES)

    p0 = nc.dram_tensor("p0", [4, 128, HP, WP], BF16, kind="ExternalInput").ap()
    p1 = nc.dram_tensor("p1", [4, 128, HP, WP], BF16, kind="ExternalInput").ap()
    p2 = nc.dram_tensor("p2", [4, 128, HP, WP], BF16, kind="ExternalInput").ap()
    wt = nc.dram_tensor("wt", [4, NS, 128, F], BF16, kind="ExternalInput").ap()
    out = nc.dram_tensor("out_t", [4, F, H, W], F32, kind="ExternalOutput").ap()

    rows_per_group = 4 * NQ  # 16
    win_rows = rows_per_group + 2  # 18

    with tile.TileContext(nc) as tc, ExitStack() as ctx:
        wpool = ctx.enter_context(tc.tile_pool(name="wts", bufs=1))
        winpool = ctx.enter_context(tc.tile_pool(name="win", bufs=win_bufs))
        pspool = ctx.enter_context(tc.tile_pool(name="ps", bufs=8, space="PSUM"))
        outpool = ctx.enter_context(tc.tile_pool(name="ob", bufs=out_bufs))

        # All 56 weight tiles resident in SBUF, loaded once.
        w_all = wpool.tile([128, 4 * NS * F], BF16)
        nc.sync.dma_start(
            w_all[:].rearrange("p (s f) -> p s f", f=F),
            wt.rearrange("j s c f -> c (j s) f"),
        )

        def body(_iv=None):
            for j in range(4):
                t0 = (j + 3) % 4
                t1 = (j + 1) % 4
                for g in range(GROUPS):
                    h0 = rows_per_group * g
                    win0 = winpool.tile([128, win_rows * WP], BF16, tag="win0")
                    nc.sync.dma_start(
                        win0[:].rearrange("p (r c) -> p r c", c=WP),
                        p0[t0, :, h0 : h0 + win_rows, :],
                    )
                    win1 = winpool.tile([128, win_rows * WP], BF16, tag="win1")
                    nc.scalar.dma_start(
                        win1[:].rearrange("p (r c) -> p r c", c=WP),
                        p1[t1, :, h0 + 1 : h0 + 1 + win_rows, :],
                    )
                    win2 = winpool.tile([128, win_rows * WP], BF16, tag="win2")
                    nc.sync.dma_start(
                        win2[:].rearrange("p (r c) -> p r c", c=WP),
                        p2[t1, :, h0 + 1 : h0 + 1 + win_rows, :],
                    )
                    w0r = win0[:].rearrange("p (r c) -> p r c", c=WP)
                    w1r = win1[:].rearrange("p (r c) -> p r c", c=WP)
                    w2r = win2[:].rearrange("p (r c) -> p r c", c=WP)

                    psums = [
                        pspool.tile([128, 512], F32, tag="ps", name=f"ps_{j}_{g}_{q}")
                        for q in range(NQ)
                    ]
                    for s in range(NS):
                        sl = (j * NS + s) * F
                        for q in range(NQ):
                            r0 = 4 * q
                            if s < 9:
                                a, v = divmod(s, 3)
                                rhs = w0r[:, r0 + a : r0 + a + 4, v : v + W]
                            elif s < 12:
                                v = s - 9
                                rhs = w1r[:, r0 : r0 + 4, v : v + W]
                            elif s == 12:
                                rhs = w2r[:, r0 + 2 : r0 + 6, 0:W]
                            else:
                                # Last tap (27 is odd) as row-tiled K=64
                                # matmuls: even quads on array rows 0-63,
                                # odd quads on rows 64-127 (tile_position
                                # auto-derived from base_partition), so
                                # adjacent quads' matmuls run concurrently.
                                if q % 2 == 0:
                                    rhs = w1r[0:64, r0 + 2 : r0 + 6, 2 : 2 + W]
                                    lhsT = w_all[0:64, sl : sl + F]
                                else:
                                    rhs = w1r[64:128, r0 + 1 : r0 + 5, 2 : 2 + W]
                                    lhsT = w_all[64:128, sl : sl + F]
                                nc.tensor.matmul(
                                    psums[q][:], lhsT, rhs, start=False, stop=True
                                )
                                continue
                            nc.tensor.matmul(
                                psums[q][:],
                                w_all[:, sl : sl + F],
                                rhs,
                                start=(s == 0),
                                stop=(s == NS - 1),
                            )
                    for q in range(NQ):
                        ot = outpool.tile([128, 512], F32, tag="ob")
                        nc.vector.tensor_copy(ot[:], psums[q][:])
                        nc.scalar.dma_start(
                            out[j, :, h0 + 4 * q : h0 + 4 * q + 4, :],
                            ot[:].rearrange("p (r c) -> p r c", c=W),
                        )

        if loop_iters > 1:
            with tc.For_i(0, loop_iters, 1) as iv:
                body(iv)
        else:
            body()

    nc.compile()
    return nc


_PROGRAM_CACHE = {}


def _get_program(loop_iters: int = 1):
    if loop_iters not in _PROGRAM_CACHE:
        _PROGRAM_CACHE[loop_iters] = build_program(loop_iters)
    return _PROGRAM_CACHE[loop_iters]


def kernel(**inputs) -> np.ndarray:
    x = np.ascontiguousarray(np.asarray(inputs["x"], dtype=np.float32))
    kern = np.ascontiguousarray(np.asarray(inputs["kernel"], dtype=np.float32))
    assert x.shape == (B, 4, H, W, CIN), x.shape
    assert kern.shape == (3, 3, 3, CIN, F), kern.shape

    nc = _get_program(1)
    in_maps = _stage_inputs(x, kern)

    last_err = None
    for _attempt in range(3):
        try:
            res = run_bass_kernel_spmd(nc, in_maps, list(range(N_CORES)))
            break
        except Exception as e:  # transient device wedge: retry
            last_err = e
    else:
        raise last_err

    stacked = np.stack([r["out_t"] for r in res.results])  # [b, j, f, h, w]
    return np.ascontiguousarray(stacked.transpose(0, 1, 3, 4, 2))


# revision 6
# speedup vs baseline: 1.0102x; 1.0102x over previous
"""nn_ConvP4 TRN2 Bass kernel: P4 group-equivariant convolution on 8 NeuronCores.

out[b,j,h,w,f] = sum_{k,a,v,c} x[b,(j+k-1)%4, h+a-1, w+v-1, c] * rot90(kernel,j)[a,v,k,c,f]
  x: [8,4,128,128,64] f32, kernel: [3,3,3,64,128] f32 -> out: [8,4,128,128,128] f32

Sharding: data-parallel over batch, one batch item per core (8 cores), kernel
weights replicated. No cross-device communication.

Device algorithm (per core / batch item): for each output group j and each
4-row output quad, accumulate 14 matmuls [K=128, M=128(F)] x [K=128, N=512]
into one PSUM bank. The 27 taps (3 kh x 3 kw x 3 group-depth) are packed two
per matmul by stacking two tap-shifted copies of x in the 128 SBUF partitions
(64 channels each):
  - 9 slots: taps (a,v,k=0) + (a,v,k=1) stacked channel-wise (tensor P0).
  - 3 slots: taps (0,v,2) + (1,v,2) via a row-shifted staging tensor P1.
  - 1 slot:  taps (2,0,2) + (2,1,2) via a column-shifted staging tensor P2.
  - 1 slot:  tap (2,2,2) alone (27 is odd), zero weights in the high half.
Inputs and weights are staged in bf16 (halves DMA traffic, full-rate PE);
accumulation is fp32 in PSUM. Measured rel error ~2.3e-3 for this 1728-term
contraction, well under the 2e-2 gate.

Host side stages channel-major, spatially zero-padded tensors so the device
only ever issues dense, large, contiguous-per-partition DMAs, and the output
is produced in [j, f, h, w] layout which the host transposes during unshard.
"""

from contextlib import ExitStack

import ml_dtypes
import numpy as np

import concourse.bacc as bacc
import concourse.tile as tile
from concourse import mybir
from concourse.bass_utils import run_bass_kernel_spmd

N_CORES = 8
B = 8
H = W = 128
CIN = 64
F = 128
HP = 132  # padded rows in staged tensors
WP = 130  # padded cols
NQ = 4  # quads per PSUM group
GROUPS = H // (4 * NQ)
NS = 14  # matmul slots per quad

F32 = mybir.dt.float32
BF16 = mybir.dt.bfloat16
NP_BF16 = ml_dtypes.bfloat16


def _stage_inputs(x: np.ndarray, kern: np.ndarray):
    """Per-core input maps from full inputs."""
    xt = np.ascontiguousarray(x.transpose(0, 1, 4, 2, 3))  # [b,g,c,h,w]
    xt = xt.astype(NP_BF16)
    P0 = np.zeros((B, 4, 128, HP, WP), NP_BF16)
    P1 = np.zeros((B, 4, 128, HP, WP), NP_BF16)
    P2 = np.zeros((B, 4, 128, HP, WP), NP_BF16)
    for t in range(4):
        P0[:, t, 0:64, 1 : H + 1, 1 : W + 1] = xt[:, t]
        P0[:, t, 64:128, 1 : H + 1, 1 : W + 1] = xt[:, (t + 1) % 4]
        P1[:, t, 0:64, 2 : H + 2, 1 : W + 1] = xt[:, t]
        P1[:, t, 64:128, 1 : H + 1, 1 : W + 1] = xt[:, t]
        P2[:, t, 0:64, 2 : H + 2, 1 : W + 1] = xt[:, t]
        P2[:, t, 64:128, 2 : H + 2, 0:W] = xt[:, t]

    Wpk = np.zeros((4, NS, 128, F), np.float32)
    for j in range(4):
        Kj = np.rot90(kern, k=j, axes=(0, 1))
        for a in range(3):
            for v in range(3):
                Wpk[j, 3 * a + v, 0:64] = Kj[a, v, 0]
                Wpk[j, 3 * a + v, 64:128] = Kj[a, v, 1]
        for v in range(3):
            Wpk[j, 9 + v, 0:64] = Kj[0, v, 2]
            Wpk[j, 9 + v, 64:128] = Kj[1, v, 2]
        Wpk[j, 12, 0:64] = Kj[2, 0, 2]
        Wpk[j, 12, 64:128] = Kj[2, 1, 2]
        Wpk[j, 13, 0:64] = Kj[2, 2, 2]
        Wpk[j, 13, 64:128] = Kj[2, 2, 2]  # duplicated for the high row-tile
    Wpk = Wpk.astype(NP_BF16)
    return [{"p0": P0[b], "p1": P1[b], "p2": P2[b], "wt": Wpk} for b in range(B)]


def build_program(loop_iters: int = 1, out_bufs: int = 6, win_bufs: int = 4):
    """Build + compile the per-core Bass program (identical on all cores)."""
    nc = bacc.Bacc("TRN2", target_bir_lowering=False, debug=False, num_devices=N_CORES)

    p0 = nc.dram_tensor("p0", [4, 128, HP, WP], BF16, kind="ExternalInput").ap()
    p1 = nc.dram_tensor("p1", [4, 128, HP, WP], BF16, kind="ExternalInput").ap()
    p2 = nc.dram_tensor("p2", [4, 128, HP, WP], BF16, kind="ExternalInput").ap()
    wt = nc.dram_tensor("wt", [4, NS, 128, F], BF16, kind="ExternalInput").ap()
    out = nc.dram_tensor("out_t", [4, F, H, W], F32, kind="ExternalOutput").ap()

    rows_per_group = 4 * NQ  # 16
    win_rows = rows_per_group + 2  # 18

    with tile.TileContext(nc) as tc, ExitStack() as ctx:
        wpool = ctx.enter_context(tc.tile_pool(name="wts", bufs=1))
        winpool = ctx.enter_context(tc.tile_pool(name="win", bufs=win_bufs))
        pspool = ctx.enter_context(tc.tile_pool(name="ps", bufs=8, space="PSUM"))
        outpool = ctx.enter_context(tc.tile_pool(name="ob", bufs=out_bufs))

        # All 56 weight tiles resident in SBUF, loaded once.
        w_all = wpool.tile([128, 4 * NS * F], BF16)
        nc.sync.dma_start(
            w_all[:].rearrange("p (s f) -> p s f", f=F),
            wt.rearrange("j s c f -> c (j s) f"),
        )

        def body(_iv=None):
            for j in range(4):
                t0 = (j + 3) % 4
                t1 = (j + 1) % 4
                for g in range(GROUPS):
                    h0 = rows_per_group * g
                    win0 = winpool.tile([128, win_rows * WP], BF16, tag="win0")
                    nc.sync.dma_start(
                        win0[:].rearrange("p (r c) -> p r c", c=WP),
                        p0[t0, :, h0 : h0 + win_rows, :],
                    )
                    win1 = winpool.tile([128, win_rows * WP], BF16, tag="win1")
                    nc.scalar.dma_start(
                        win1[:].rearrange("p (r c) -> p r c", c=WP),
                        p1[t1, :, h0 + 1 : h0 + 1 + win_rows, :],
                    )
                    win2 = winpool.tile([128, win_rows * WP], BF16, tag="win2")
                    nc.sync.dma_start(
                        win2[:].rearrange("p (r c) -> p r c", c=WP),
                        p2[t1, :, h0 + 1 : h0 + 1 + win_rows, :],
                    )
                    w0r = win0[:].rearrange("p (r c) -> p r c", c=WP)
                    w1r = win1[:].rearrange("p (r c) -> p r c", c=WP)
                    w2r = win2[:].rearrange("p (r c) -> p r c", c=WP)

                    psums = [
                        pspool.tile([128, 512], F32, tag="ps", name=f"ps_{j}_{g}_{q}")
                        for q in range(NQ)
                    ]
                    for s in range(NS):
                        sl = (j * NS + s) * F
                        for q in range(NQ):
                            r0 = 4 * q
                            if s < 9:
                                a, v = divmod(s, 3)
                                rhs = w0r[:, r0 + a : r0 + a + 4, v : v + W]
                            elif s < 12:
                                v = s - 9
                                rhs = w1r[:, r0 : r0 + 4, v : v + W]
                            elif s == 12:
                                rhs = w2r[:, r0 + 2 : r0 + 6, 0:W]
                            else:
                                # Last tap (27 is odd) as row-tiled K=64
                                # matmuls: even quads on array rows 0-63,
                                # odd quads on rows 64-127 (tile_position
                                # auto-derived from base_partition), so
                                # adjacent quads' matmuls run concurrently.
                                if q % 2 == 0:
                                    rhs = w1r[0:64, r0 + 2 : r0 + 6, 2 : 2 + W]
                                    lhsT = w_all[0:64, sl : sl + F]
                                else:
                                    rhs = w1r[64:128, r0 + 1 : r0 + 5, 2 : 2 + W]
                                    lhsT = w_all[64:128, sl : sl + F]
                                nc.tensor.matmul(
                                    psums[q][:], lhsT, rhs, start=False, stop=True
                                )
                                continue
                            nc.tensor.matmul(
                                psums[q][:],
                                w_all[:, sl : sl + F],
                                rhs,
                                start=(s == 0),
                                stop=(s == NS - 1),
                            )
                    for q in range(NQ):
                        ot = outpool.tile([128, 512], F32, tag="ob")
                        nc.vector.tensor_copy(ot[:], psums[q][:])
                        # balance store traffic across the two HWDGE queues
                        dma_eng = nc.scalar if q % 2 == 0 else nc.sync
                        dma_eng.dma_start(
                            out[j, :, h0 + 4 * q : h0 + 4 * q + 4, :],
                            ot[:].rearrange("p (r c) -> p r c", c=W),
                        )

        if loop_iters > 1:
            with tc.For_i(0, loop_iters, 1) as iv:
                body(iv)
        else:
            body()

    nc.compile()
    return nc


_PROGRAM_CACHE = {}


def _get_program(loop_iters: int = 1):
    if loop_iters not in _PROGRAM_CACHE:
        _PROGRAM_CACHE[loop_iters] = build_program(loop_iters)
    return _PROGRAM_CACHE[loop_iters]


def kernel(**inputs) -> np.ndarray:
    x = np.ascontiguousarray(np.asarray(inputs["x"], dtype=np.float32))
    kern = np.ascontiguousarray(np.asarray(inputs["kernel"], dtype=np.float32))
    assert x.shape == (B, 4, H, W, CIN), x.shape
    assert kern.shape == (3, 3, 3, CIN, F), kern.shape

    nc = _get_program(1)
    in_maps = _stage_inputs(x, kern)

    last_err = None
    for _attempt in range(3):
        try:
            res = run_bass_kernel_spmd(nc, in_maps, list(range(N_CORES)))
            break
        except Exception as e:  # transient device wedge: retry
            last_err = e
    else:
        raise last_err

    stacked = np.stack([r["out_t"] for r in res.results])  # [b, j, f, h, w]
    return np.ascontiguousarray(stacked.transpose(0, 1, 3, 4, 2))
